# revision 28
# baseline (speedup 1.0000x reference)
"""Block-sparse linear layer (x @ (mask*W).T + bias) on 8 TRN2 NeuronCores.

Strategy: data-parallel over batch rows. Each core gets 1024 rows of x
(transposed to [k, m] on host, cast to bf16), the packed kept weight
blocks (bf16), and bias. On-device: out.T tile [o=128, m=1024] accumulates
in PSUM over the 16 kept k-subtiles (k-subtile = 128 rows), with W tiles
stationary and x slabs moving. PSUM is evicted through the vector/scalar
engines with the per-partition bias add fused, then DMA'd out. The host
reassembles the full [8192, 4096] fp32 output.

Startup is latency-tuned: x streams on the Sync HWDGE queue with the
first k-subtiles in single-subtile DMAs, weights stream on the Scalar
HWDGE queue with the first four o-tiles in quarter-tile chunks, and PE
warmup matmuls bridge the gap until the first real tiles land (keeps the
HAM clock gate open with no idle window).
"""

import sys
import types

import numpy as np
import ml_dtypes

BATCH = 8192
SIZE = 4096
NB = 16
BLOCK = 256
NCORES = 8
MC = BATCH // NCORES  # 1024 rows per core
P = 128
KS = SIZE // P  # 32 k-subtiles
OT = SIZE // P  # 32 o-tiles
MM_N = 512  # moving free dim per matmul

# x DMA group sizes (in k-subtiles), in first-use order: fine-grained at
# startup so the first matmul can begin ~1us after the first transfer.
X_GROUP_SIZES = [1, 1, 2, 2, 2, 4, 4, 4, 4, 4, 4]
assert sum(X_GROUP_SIZES) == KS

_BUILD_CACHE = {}


def _install_ntff_hook():
    # Register the axon NTFF profiling hook if the image's antenv lacks it.
    if "antenv.axon_hooks" in sys.modules:
        return
    try:
        from trn_agent_boot.trn_boot import _ntff_profile_via_ctypes

        hook = _ntff_profile_via_ctypes("/opt/axon/libaxon_pjrt.so")
        mod = types.ModuleType("antenv.axon_hooks")
        mod.get_axon_ntff_profile_hook = lambda: hook
        sys.modules["antenv.axon_hooks"] = mod
    except Exception:
        pass


def _block_keep_from_mask(mask):
    """Return [NB, NB] bool of kept blocks if mask is block-constant, else None."""
    m4 = np.asarray(mask).reshape(NB, BLOCK, NB, BLOCK)
    keep = m4[:, 0, :, 0]
    uniform = np.all(m4 == keep[:, None, :, None])
    return keep if uniform else None


def _ks_lists(keep):
    """Per o-tile (128 outputs) list of kept k-subtile indices, padded to
    a uniform length (padding points at subtile 0 with zero weights).

    Each tile's list is ordered by global first-use (consumption) order,
    so a prefix DMA of the packed weight tile covers exactly the chunks
    the PE consumes first."""
    lists = []
    for t in range(OT):
        i = (t * P) // BLOCK  # o-block row
        ks = []
        for j in range(NB):
            if keep[i, j]:
                base = (j * BLOCK) // P
                ks.extend(range(base, base + BLOCK // P))
        lists.append(ks)
    # global first-use order across tiles
    order = []
    for l in lists:
        for ks in l:
            if ks not in order:
                order.append(ks)
    rank = {ks: r for r, ks in enumerate(order)}
    lists = [sorted(l, key=lambda ks: rank[ks]) for l in lists]
    n_sub = max(1, max(len(l) for l in lists))
    padded = tuple(tuple(l + [-1] * (n_sub - len(l))) for l in lists)
    return padded, n_sub


def _build(ks_lists, n_sub):
    import concourse.mybir as mybir
    import concourse.tile as tile
    from concourse import bacc

    bf16, f32 = mybir.dt.bfloat16, mybir.dt.float32
    nc = bacc.Bacc("TRN2", target_bir_lowering=False)
    xt_d = nc.declare_dram_parameter("xt", [P, KS, MC], bf16, isOutput=False)
    wt_d = nc.declare_dram_parameter("wt", [OT, P, n_sub, P], bf16, isOutput=False)
    bias_d = nc.declare_dram_parameter("biast", [P, OT], f32, isOutput=False)
    out_d = nc.declare_dram_parameter("out", [OT, P, MC], f32, isOutput=True)

    # x DMA issue order: k-subtiles in order of first use across o-tiles.
    ks_order_raw = []
    for t in range(OT):
        for ks in ks_lists[t]:
            if ks >= 0 and ks not in ks_order_raw:
                ks_order_raw.append(ks)
    for ks in range(KS):
        if ks not in ks_order_raw:
            ks_order_raw.append(ks)

    # Chop into contiguous-range groups following first-use order, with
    # X_GROUP_SIZES granularity. Each group is one DMA; delivery of a
    # group makes all its subtiles available at once, so the effective
    # arrival order is group-major.
    x_groups = []  # (lo, n)
    remaining = list(ks_order_raw)
    for size in X_GROUP_SIZES:
        # take the first `size` not-yet-grouped subtiles, extend to a
        # contiguous range covering them
        take = remaining[:size]
        lo, hi = min(take), max(take) + 1
        # widen to contiguous range within remaining
        while hi - lo < size and (
            lo - 1 in remaining or hi in remaining
        ):
            if hi in remaining:
                hi += 1
            elif lo - 1 in remaining:
                lo -= 1
        grp = list(range(lo, hi))
        assert all(g in remaining for g in grp), (grp, remaining)
        x_groups.append((lo, hi - lo))
        remaining = [r for r in remaining if not (lo <= r < hi)]
    assert not remaining, remaining
    ks_order = [lo + i for lo, n in x_groups for i in range(n)]

    W_POOL_BUFS = 8

    with tile.TileContext(nc) as tc:
        with (
            tc.tile_pool(name="const", bufs=1) as const_pool,
            tc.tile_pool(name="xpool", bufs=1) as xpool,
            tc.tile_pool(name="wpool", bufs=W_POOL_BUFS) as wpool,
            tc.tile_pool(name="opool", bufs=5) as opool,
            tc.tile_pool(name="psum", bufs=4, space="PSUM") as psum_pool,
        ):
            # Warm the PE clock (HAM un-throttles after ~3.4us of sustained
            # matmul activity) with dummy matmuls on zeroed SBUF while the
            # first x/W DMAs are still in flight. The memset runs on gpsimd
            # (idle at body start) so the warmups begin as early as possible.
            warm = const_pool.tile([P, MM_N], bf16, name="warm")
            nc.gpsimd.memset(warm[:], 0)
            warm_ps = psum_pool.tile([P, MM_N], f32, name="warm_ps", tag="ps")
            N_WARM = 12
            for i in range(N_WARM):
                nc.tensor.matmul(
                    warm_ps[:],
                    lhsT=warm[:, 0:P],
                    rhs=warm[:],
                    start=(i == 0),
                    stop=(i == N_WARM - 1),
                )

            w_tiles = {}

            def w_alloc(t):
                w_tiles[t] = wpool.tile([P, n_sub, P], bf16, name="w_tile")

            def w_dma(t, engine, subs=None):
                if t not in w_tiles:
                    w_alloc(t)
                w = w_tiles[t]
                lo, hi = subs if subs is not None else (0, n_sub)
                hi = min(hi, n_sub)
                if lo >= hi:
                    return
                engine.dma_start(out=w[:, lo:hi, :], in_=wt_d[t, :, lo:hi, :])

            x_ap = {}

            def x_dma(gi, split=1, engine=None):
                engine = engine or nc.sync
                lo, n = x_groups[gi]
                xg = xpool.tile([P, n, MC], bf16, name=f"x_g{gi}", uniquify=False)
                for si in range(split):
                    sl = slice(si * MC // split, (si + 1) * MC // split)
                    engine.dma_start(out=xg[:, :, sl], in_=xt_d[:, lo : lo + n, sl])
                for off in range(n):
                    x_ap[lo + off] = xg[:, off, :]

            # Startup DMA issue order, tuned for time-to-first-matmul and
            # ramp bandwidth. Sync: x groups, finest first (late groups are
            # deferred into the pair loop to keep startup bandwidth for the
            # first tiles). Scalar (HWDGE): W for the first four o-tiles in
            # consumption-order slices, finest first.
            x_dma(0, split=2)
            x_dma(1)
            step = (n_sub + 3) // 4
            half_step = (step + 1) // 2
            for t in range(4):
                w_dma(t, nc.scalar, subs=(0, half_step))
            for t in range(4):
                w_dma(t, nc.scalar, subs=(half_step, step))
            x_dma(2)
            x_dma(3)
            for q in range(1, 4):
                for t in range(4):
                    w_dma(t, nc.scalar, subs=(q * step, (q + 1) * step))
            x_dma(4)
            x_dma(5)
            bias_tile = const_pool.tile([P, OT], f32)
            nc.sync.dma_start(out=bias_tile[:], in_=bias_d[:])
            x_dma(6)
            x_dma(7)
            w_dma(4, nc.scalar)
            w_dma(5, nc.scalar)

            def emit_block(ts, interleave):
                """Emit the accumulation + eviction for o-tiles `ts`,
                chunk-major across the tiles (each arriving x chunk is
                consumed by every tile that uses it — PE executes strictly
                in order, so this is what absorbs DMA latency)."""
                ps = {t: psum_pool.tile([P, MC], f32, name="ps") for t in ts}
                sets = {t: {ks: s for s, ks in enumerate(ks_lists[t]) if ks >= 0} for t in ts}
                for t in ts:
                    if not sets[t]:  # fully-masked o-tile: zero the PSUM
                        sets[t] = {ks_order[0]: 0}
                n_done = {t: 0 for t in ts}
                order = [(c, t) for c in ks_order for t in ts if c in sets[t]]
                for c, t in order:
                    s = sets[t][c]
                    first = n_done[t] == 0
                    n_done[t] += 1
                    last = n_done[t] == len(sets[t])
                    for h in range(MC // MM_N):
                        nc.tensor.matmul(
                            ps[t][:, h * MM_N : (h + 1) * MM_N],
                            lhsT=w_tiles[t][:, s, :],
                            rhs=x_ap[c][:, h * MM_N : (h + 1) * MM_N],
                            start=first,
                            stop=last,
                        )
                for t in ts:
                    _evict(t, ps)

            def emit_tail_block(ts):
                """Final o-tiles, half-major: each [128, 512] half is a
                separate accumulation group; a completed half is evicted
                and DMA'd while the next half's matmuls run, so only the
                very last half's eviction trails the final matmul."""
                ps = {t: psum_pool.tile([P, MC], f32, name="ps") for t in ts}
                o_tiles = {t: opool.tile([P, MC], f32, name="o_tile") for t in ts}
                for t in ts:
                    chunks = [c for c in ks_lists[t] if c >= 0] or [ks_order[0]]
                    for h in range(MC // MM_N):
                        sl = slice(h * MM_N, (h + 1) * MM_N)
                        for ci, c in enumerate(chunks):
                            nc.tensor.matmul(
                                ps[t][:, sl],
                                lhsT=w_tiles[t][:, sets_of(t, c), :],
                                rhs=x_ap[c][:, sl],
                                start=(ci == 0),
                                stop=(ci == len(chunks) - 1),
                            )
                        # evict this half in parallel quarters, one out-DMA
                        q = MM_N // 2
                        lo = h * MM_N
                        nc.vector.tensor_scalar_add(
                            o_tiles[t][:, lo : lo + q],
                            ps[t][:, lo : lo + q],
                            bias_tile[:, t : t + 1],
                        )
                        nc.scalar.add(
                            o_tiles[t][:, lo + q : lo + 2 * q],
                            ps[t][:, lo + q : lo + 2 * q],
                            bias_tile[:, t : t + 1],
                        )
                        eng = nc.scalar if h == 1 else nc.sync
                        eng.dma_start(
                            out=out_d[t, :, sl], in_=o_tiles[t][:, sl]
                        )

            def sets_of(t, c):
                for s, ks in enumerate(ks_lists[t]):
                    if ks == c:
                        return s
                return 0

            def _evict(t, ps):
                # PSUM -> SBUF with fused per-partition bias add; both
                # halves run in parallel (vector + scalar), then one DMA
                # moves the whole tile out.
                o_tile = opool.tile([P, MC], f32, name="o_tile")
                half = MC // 2
                nc.vector.tensor_scalar_add(
                    o_tile[:, 0:half], ps[t][:, 0:half], bias_tile[:, t : t + 1]
                )
                nc.scalar.add(
                    o_tile[:, half:MC], ps[t][:, half:MC], bias_tile[:, t : t + 1]
                )
                nc.sync.dma_start(out=out_d[t, :, :], in_=o_tile[:])

            # First four o-tiles as one interleaved block (their k-chunk
            # sets overlap heavily, maximizing PE work per arriving byte
            # during the x load); middle o-tiles pair-wise; last pair
            # half-major so the final eviction trails by <2us.
            emit_block((0, 1, 2, 3), interleave=True)
            for pair in range(2, OT // 2):
                ts = (2 * pair, 2 * pair + 1)
                # deferred bulk x groups: issued while block 0 computes,
                # after the startup-critical transfers have drained
                if pair == 2:
                    for gi in range(8, len(x_groups)):
                        x_dma(gi)
                for t in ts:
                    if t >= 6:
                        w_dma(t, nc.scalar if t % 2 == 0 else nc.sync)
                if pair != OT // 2 - 1:
                    emit_block(ts, interleave=True)
                else:
                    emit_tail_block(ts)
    nc.compile()
    return nc


def _get_kernel(ks_lists, n_sub):
    key = (ks_lists, n_sub)
    if key not in _BUILD_CACHE:
        _BUILD_CACHE[key] = _build(ks_lists, n_sub)
    return _BUILD_CACHE[key]


def kernel(x, weight, bias, mask, _trace=False):
    from concourse.bass_utils import run_bass_kernel_spmd

    _install_ntff_hook()

    x = np.asarray(x)
    weight = np.asarray(weight)
    bias = np.asarray(bias, dtype=np.float32)
    keep = _block_keep_from_mask(mask)
    if keep is None:
        # Mask not block-constant: fall back to a dense schedule with the
        # element-masked weights and every k-subtile kept.
        weight = np.where(np.asarray(mask), weight, 0.0).astype(np.float32)
        keep = np.ones((NB, NB), dtype=bool)
    ks_lists, n_sub = _ks_lists(keep)

    nc = _get_kernel(ks_lists, n_sub)

    # Pack weights: wt[t, p, s, q] = W[t*P + q, ks*P + p] for kept subtile ks.
    w4 = weight.reshape(OT, P, KS, P)  # [t, q, ks, p]
    wt = np.zeros((OT, P, n_sub, P), dtype=ml_dtypes.bfloat16)
    for t in range(OT):
        idx = [ks for ks in ks_lists[t]]
        valid = [s for s, ks in enumerate(idx) if ks >= 0]
        sel = w4[t][:, [idx[s] for s in valid], :]  # [q, s_valid, p]
        wt[t][:, valid, :] = sel.transpose(2, 1, 0).astype(ml_dtypes.bfloat16)

    biast = np.ascontiguousarray(
        bias.reshape(OT, P).T, dtype=np.float32
    )  # [P, OT]

    in_maps = []
    for c in range(NCORES):
        xc = x[c * MC : (c + 1) * MC, :]  # [MC, SIZE] fp32
        xt = np.ascontiguousarray(
            xc.reshape(MC, KS, P).transpose(2, 1, 0)
        ).astype(ml_dtypes.bfloat16)  # [P, KS, MC]
        in_maps.append({"xt": xt, "wt": wt, "biast": biast})

    res = run_bass_kernel_spmd(nc, in_maps, list(range(NCORES)), trace=_trace)

    out = np.empty((BATCH, SIZE), dtype=np.float32)
    for c in range(NCORES):
        o = res.results[c]["out"]  # [OT, P, MC]
        out[c * MC : (c + 1) * MC, :] = o.reshape(SIZE, MC).T
    if _trace:
        return out, res
    return out


# revision 30
# speedup vs baseline: 1.1766x; 1.1766x over previous
"""Block-sparse linear layer (x @ (mask*W).T + bias) on 8 TRN2 NeuronCores.

Strategy: data-parallel over batch rows. Each core gets 1024 rows of x
(transposed to [k, m] on host, cast to bf16), the packed kept weight
blocks (bf16), and bias. On-device: out.T tile [o=128, m=1024] accumulates
in PSUM over the 16 kept k-subtiles (k-subtile = 128 rows), with W tiles
stationary and x slabs moving. PSUM is evicted through the vector/scalar
engines with the per-partition bias add fused, then DMA'd out. The host
reassembles the full [8192, 4096] fp32 output.

Startup is latency-tuned: x streams on the Sync HWDGE queue with the
first k-subtiles in single-subtile DMAs, weights stream on the Scalar
HWDGE queue with the first four o-tiles in quarter-tile chunks, and PE
warmup matmuls bridge the gap until the first real tiles land (keeps the
HAM clock gate open with no idle window).
"""

import sys
import types

import numpy as np
import ml_dtypes

BATCH = 8192
SIZE = 4096
NB = 16
BLOCK = 256
NCORES = 8
MC = BATCH // NCORES  # 1024 rows per core
P = 128
KS = SIZE // P  # 32 k-subtiles
OT = SIZE // P  # 32 o-tiles
MM_N = 512  # moving free dim per matmul

# x DMA group sizes (in k-subtiles), in first-use order: fine-grained at
# startup so the first matmul can begin ~1us after the first transfer.
X_GROUP_SIZES = [1, 1, 2, 2, 2, 4, 4, 4, 4, 4, 4]
assert sum(X_GROUP_SIZES) == KS

_BUILD_CACHE = {}


def _install_ntff_hook():
    # Register the axon NTFF profiling hook if the image's antenv lacks it.
    if "antenv.axon_hooks" in sys.modules:
        return
    try:
        from trn_agent_boot.trn_boot import _ntff_profile_via_ctypes

        hook = _ntff_profile_via_ctypes("/opt/axon/libaxon_pjrt.so")
        mod = types.ModuleType("antenv.axon_hooks")
        mod.get_axon_ntff_profile_hook = lambda: hook
        sys.modules["antenv.axon_hooks"] = mod
    except Exception:
        pass


def _block_keep_from_mask(mask):
    """Return [NB, NB] bool of kept blocks if mask is block-constant, else None."""
    m4 = np.asarray(mask).reshape(NB, BLOCK, NB, BLOCK)
    keep = m4[:, 0, :, 0]
    uniform = np.all(m4 == keep[:, None, :, None])
    return keep if uniform else None


def _ks_lists(keep):
    """Per o-tile (128 outputs) list of kept k-subtile indices, padded to
    a uniform length (padding points at subtile 0 with zero weights).

    Each tile's list is ordered by global first-use (consumption) order,
    so a prefix DMA of the packed weight tile covers exactly the chunks
    the PE consumes first."""
    lists = []
    for t in range(OT):
        i = (t * P) // BLOCK  # o-block row
        ks = []
        for j in range(NB):
            if keep[i, j]:
                base = (j * BLOCK) // P
                ks.extend(range(base, base + BLOCK // P))
        lists.append(ks)
    # global first-use order across tiles
    order = []
    for l in lists:
        for ks in l:
            if ks not in order:
                order.append(ks)
    rank = {ks: r for r, ks in enumerate(order)}
    lists = [sorted(l, key=lambda ks: rank[ks]) for l in lists]
    n_sub = max(1, max(len(l) for l in lists))
    padded = tuple(tuple(l + [-1] * (n_sub - len(l))) for l in lists)
    return padded, n_sub


def _build(ks_lists, n_sub):
    import concourse.mybir as mybir
    import concourse.tile as tile
    from concourse import bacc

    bf16, f32 = mybir.dt.bfloat16, mybir.dt.float32
    nc = bacc.Bacc("TRN2", target_bir_lowering=False)
    xt_d = nc.declare_dram_parameter("xt", [P, KS, MC], bf16, isOutput=False)
    wt_d = nc.declare_dram_parameter("wt", [OT, P, n_sub, P], bf16, isOutput=False)
    bias_d = nc.declare_dram_parameter("biast", [P, OT], f32, isOutput=False)
    out_d = nc.declare_dram_parameter("out", [OT, P, MC], f32, isOutput=True)

    # x DMA issue order: k-subtiles in order of first use across o-tiles.
    ks_order_raw = []
    for t in range(OT):
        for ks in ks_lists[t]:
            if ks >= 0 and ks not in ks_order_raw:
                ks_order_raw.append(ks)
    for ks in range(KS):
        if ks not in ks_order_raw:
            ks_order_raw.append(ks)

    # Chop into contiguous-range groups following first-use order, with
    # X_GROUP_SIZES granularity. Each group is one DMA; delivery of a
    # group makes all its subtiles available at once, so the effective
    # arrival order is group-major.
    x_groups = []  # (lo, n)
    remaining = list(ks_order_raw)
    for size in X_GROUP_SIZES:
        # take the first `size` not-yet-grouped subtiles, extend to a
        # contiguous range covering them
        take = remaining[:size]
        lo, hi = min(take), max(take) + 1
        # widen to contiguous range within remaining
        while hi - lo < size and (
            lo - 1 in remaining or hi in remaining
        ):
            if hi in remaining:
                hi += 1
            elif lo - 1 in remaining:
                lo -= 1
        grp = list(range(lo, hi))
        assert all(g in remaining for g in grp), (grp, remaining)
        x_groups.append((lo, hi - lo))
        remaining = [r for r in remaining if not (lo <= r < hi)]
    assert not remaining, remaining
    ks_order = [lo + i for lo, n in x_groups for i in range(n)]

    W_POOL_BUFS = 8

    with tile.TileContext(nc) as tc:
        with (
            tc.tile_pool(name="const", bufs=1) as const_pool,
            tc.tile_pool(name="xpool", bufs=1) as xpool,
            tc.tile_pool(name="wpool", bufs=W_POOL_BUFS) as wpool,
            tc.tile_pool(name="opool", bufs=5) as opool,
            tc.tile_pool(name="psum", bufs=4, space="PSUM") as psum_pool,
        ):
            # Warm the PE clock (HAM un-throttles after ~3.4us of sustained
            # matmul activity) with dummy matmuls on zeroed SBUF while the
            # first x/W DMAs are still in flight. The memset runs on gpsimd
            # (idle at body start) so the warmups begin as early as possible.
            warm = const_pool.tile([P, MM_N], bf16, name="warm")
            nc.gpsimd.memset(warm[:], 0)
            warm_ps = psum_pool.tile([P, MM_N], f32, name="warm_ps", tag="ps")
            N_WARM = 12
            for i in range(N_WARM):
                nc.tensor.matmul(
                    warm_ps[:],
                    lhsT=warm[:, 0:P],
                    rhs=warm[:],
                    start=(i == 0),
                    stop=(i == N_WARM - 1),
                )

            w_tiles = {}

            def w_alloc(t):
                w_tiles[t] = wpool.tile([P, n_sub, P], bf16, name="w_tile")

            def w_dma(t, engine, subs=None):
                if t not in w_tiles:
                    w_alloc(t)
                w = w_tiles[t]
                lo, hi = subs if subs is not None else (0, n_sub)
                hi = min(hi, n_sub)
                if lo >= hi:
                    return
                engine.dma_start(out=w[:, lo:hi, :], in_=wt_d[t, :, lo:hi, :])

            x_ap = {}

            def x_dma(gi, split=1, engine=None):
                engine = engine or nc.sync
                lo, n = x_groups[gi]
                xg = xpool.tile([P, n, MC], bf16, name=f"x_g{gi}", uniquify=False)
                for si in range(split):
                    sl = slice(si * MC // split, (si + 1) * MC // split)
                    engine.dma_start(out=xg[:, :, sl], in_=xt_d[:, lo : lo + n, sl])
                for off in range(n):
                    x_ap[lo + off] = xg[:, off, :]

            # Startup DMA issue order, tuned for time-to-first-matmul and
            # ramp bandwidth. Sync: x groups, finest first (late groups are
            # deferred into the pair loop to keep startup bandwidth for the
            # first tiles). Scalar (HWDGE): W for the first four o-tiles in
            # consumption-order slices, finest first.
            x_dma(0, split=2)
            step = (n_sub + 3) // 4
            half_step = (step + 1) // 2
            # First slices of the four startup W tiles split across BOTH
            # HWDGE queue sets so they arrive as two parallel pairs, in
            # PE consumption order (t0,t1 on scalar; t2,t3 on sync).
            w_dma(0, nc.scalar, subs=(0, half_step))
            w_dma(1, nc.scalar, subs=(0, half_step))
            w_dma(2, nc.sync, subs=(0, half_step))
            w_dma(3, nc.sync, subs=(0, half_step))
            x_dma(1)
            for t in range(4):
                w_dma(t, nc.scalar, subs=(half_step, step))
            x_dma(2)
            x_dma(3)
            for q in range(1, 4):
                for t in range(4):
                    w_dma(t, nc.scalar, subs=(q * step, (q + 1) * step))
            x_dma(4)
            x_dma(5)
            bias_tile = const_pool.tile([P, OT], f32)
            nc.sync.dma_start(out=bias_tile[:], in_=bias_d[:])
            x_dma(6)
            x_dma(7)
            w_dma(4, nc.scalar)
            w_dma(5, nc.scalar)

            def emit_block(ts, interleave):
                """Emit the accumulation + eviction for o-tiles `ts`,
                chunk-major across the tiles (each arriving x chunk is
                consumed by every tile that uses it — PE executes strictly
                in order, so this is what absorbs DMA latency)."""
                ps = {t: psum_pool.tile([P, MC], f32, name="ps") for t in ts}
                sets = {t: {ks: s for s, ks in enumerate(ks_lists[t]) if ks >= 0} for t in ts}
                for t in ts:
                    if not sets[t]:  # fully-masked o-tile: zero the PSUM
                        sets[t] = {ks_order[0]: 0}
                n_done = {t: 0 for t in ts}
                order = [(c, t) for c in ks_order for t in ts if c in sets[t]]
                for c, t in order:
                    s = sets[t][c]
                    first = n_done[t] == 0
                    n_done[t] += 1
                    last = n_done[t] == len(sets[t])
                    for h in range(MC // MM_N):
                        nc.tensor.matmul(
                            ps[t][:, h * MM_N : (h + 1) * MM_N],
                            lhsT=w_tiles[t][:, s, :],
                            rhs=x_ap[c][:, h * MM_N : (h + 1) * MM_N],
                            start=first,
                            stop=last,
                        )
                for t in ts:
                    _evict(t, ps)

            def emit_tail_block(ts):
                """Final o-tiles, half-major: each [128, 512] half is a
                separate accumulation group; a completed half is evicted
                and DMA'd while the next half's matmuls run, so only the
                very last half's eviction trails the final matmul."""
                ps = {t: psum_pool.tile([P, MC], f32, name="ps") for t in ts}
                o_tiles = {t: opool.tile([P, MC], f32, name="o_tile") for t in ts}
                for t in ts:
                    chunks = [c for c in ks_lists[t] if c >= 0] or [ks_order[0]]
                    for h in range(MC // MM_N):
                        sl = slice(h * MM_N, (h + 1) * MM_N)
                        for ci, c in enumerate(chunks):
                            nc.tensor.matmul(
                                ps[t][:, sl],
                                lhsT=w_tiles[t][:, sets_of(t, c), :],
                                rhs=x_ap[c][:, sl],
                                start=(ci == 0),
                                stop=(ci == len(chunks) - 1),
                            )
                        # evict this half in parallel quarters, one out-DMA
                        q = MM_N // 2
                        lo = h * MM_N
                        nc.vector.tensor_scalar_add(
                            o_tiles[t][:, lo : lo + q],
                            ps[t][:, lo : lo + q],
                            bias_tile[:, t : t + 1],
                        )
                        nc.scalar.add(
                            o_tiles[t][:, lo + q : lo + 2 * q],
                            ps[t][:, lo + q : lo + 2 * q],
                            bias_tile[:, t : t + 1],
                        )
                        nc.sync.dma_start(
                            out=out_d[t, :, sl], in_=o_tiles[t][:, sl]
                        )

            def sets_of(t, c):
                for s, ks in enumerate(ks_lists[t]):
                    if ks == c:
                        return s
                return 0

            def _evict(t, ps):
                # PSUM -> SBUF with fused per-partition bias add; both
                # halves run in parallel (vector + scalar), then one DMA
                # moves the whole tile out.
                o_tile = opool.tile([P, MC], f32, name="o_tile")
                half = MC // 2
                nc.vector.tensor_scalar_add(
                    o_tile[:, 0:half], ps[t][:, 0:half], bias_tile[:, t : t + 1]
                )
                nc.scalar.add(
                    o_tile[:, half:MC], ps[t][:, half:MC], bias_tile[:, t : t + 1]
                )
                nc.sync.dma_start(out=out_d[t, :, :], in_=o_tile[:])

            # First four o-tiles as one interleaved block (their k-chunk
            # sets overlap heavily, maximizing PE work per arriving byte
            # during the x load); middle o-tiles pair-wise; last pair
            # half-major so the final eviction trails by <2us.
            emit_block((0, 1, 2, 3), interleave=True)
            for pair in range(2, OT // 2):
                ts = (2 * pair, 2 * pair + 1)
                # deferred bulk x groups: issued while block 0 computes,
                # after the startup-critical transfers have drained
                if pair == 2:
                    for gi in range(8, len(x_groups)):
                        x_dma(gi)
                for t in ts:
                    if t >= 6:
                        w_dma(t, nc.scalar if t % 2 == 0 else nc.sync)
                if pair != OT // 2 - 1:
                    emit_block(ts, interleave=True)
                else:
                    emit_tail_block(ts)
    nc.compile()
    return nc


def _get_kernel(ks_lists, n_sub):
    key = (ks_lists, n_sub)
    if key not in _BUILD_CACHE:
        _BUILD_CACHE[key] = _build(ks_lists, n_sub)
    return _BUILD_CACHE[key]


def kernel(x, weight, bias, mask, _trace=False):
    from concourse.bass_utils import run_bass_kernel_spmd

    _install_ntff_hook()

    x = np.asarray(x)
    weight = np.asarray(weight)
    bias = np.asarray(bias, dtype=np.float32)
    keep = _block_keep_from_mask(mask)
    if keep is None:
        # Mask not block-constant: fall back to a dense schedule with the
        # element-masked weights and every k-subtile kept.
        weight = np.where(np.asarray(mask), weight, 0.0).astype(np.float32)
        keep = np.ones((NB, NB), dtype=bool)
    ks_lists, n_sub = _ks_lists(keep)

    nc = _get_kernel(ks_lists, n_sub)

    # Pack weights: wt[t, p, s, q] = W[t*P + q, ks*P + p] for kept subtile ks.
    w4 = weight.reshape(OT, P, KS, P)  # [t, q, ks, p]
    wt = np.zeros((OT, P, n_sub, P), dtype=ml_dtypes.bfloat16)
    for t in range(OT):
        idx = [ks for ks in ks_lists[t]]
        valid = [s for s, ks in enumerate(idx) if ks >= 0]
        sel = w4[t][:, [idx[s] for s in valid], :]  # [q, s_valid, p]
        wt[t][:, valid, :] = sel.transpose(2, 1, 0).astype(ml_dtypes.bfloat16)

    biast = np.ascontiguousarray(
        bias.reshape(OT, P).T, dtype=np.float32
    )  # [P, OT]

    in_maps = []
    for c in range(NCORES):
        xc = x[c * MC : (c + 1) * MC, :]  # [MC, SIZE] fp32
        xt = np.ascontiguousarray(
            xc.reshape(MC, KS, P).transpose(2, 1, 0)
        ).astype(ml_dtypes.bfloat16)  # [P, KS, MC]
        in_maps.append({"xt": xt, "wt": wt, "biast": biast})

    res = run_bass_kernel_spmd(nc, in_maps, list(range(NCORES)), trace=_trace)

    out = np.empty((BATCH, SIZE), dtype=np.float32)
    for c in range(NCORES):
        o = res.results[c]["out"]  # [OT, P, MC]
        out[c * MC : (c + 1) * MC, :] = o.reshape(SIZE, MC).T
    if _trace:
        return out, res
    return out
